# revision 68
# baseline (speedup 1.0000x reference)
"""Trainium2 Bass kernel for the contrastive loss problem.

Math reformulation of the reference (no [N, 2N-1] scatter needed):
  lse_i = log( exp(pos_val_i) + sum_{j in neg} exp(S_ij) + (2N-2-num_neg_i) )
  loss  = mean_i (lse_i - pos_val_i)
with S = (cos + 1) * 0.25, cos from row-normalized embeddings.

Sharding uses the Gram matrix's symmetry: core c computes only the
[512, 512*5] strip of exp(S) pairing its rows with block-columns
{c, c+1, .., c+4} (mod 8). Columns are pre-rotated on the host so the
program is identical on every core (SPMD). The device does ONLY the
heavy O(N^2 D/8) work per core: stream et in, 80 fp8-DoubleRow gram
matmuls (K=256 per op, x16-prescaled unit rows), exp on ACT, and
stream the 20 exp tiles back out — the label masking and the row /
column sums are O(N^2/8) host adds on the shipped tiles, overlapped
for free since the DMA engines idle once the input stream ends.
Distance-4 blocks are computed by both endpoint cores.

Schedule: the first row chunk is emitted k2-major so the PE consumes
the et pair-DMAs as they stream in; later tiles are j-inner
(back-to-back matmuls into one PSUM bank hold the full PE clock);
warmup matmuls ramp the PE clock during the DMA wait; early exp tiles
go out on the sync ring whose FIFO keeps them behind the input pairs;
the final block is split in two 256-wide halves so the serial
exp->DMA tail is short.

Host: norms, fp8 cast, rotation, first-positive dot products (O(N*D)),
masking + row/column sums of the exp tiles, final assembly.
"""

import sys

sys.path.insert(0, "/opt/trn_rl_repo")

from contextlib import ExitStack

import ml_dtypes
import numpy as np

import concourse.bacc as bacc
import concourse.tile as tile
from concourse import mybir
from concourse.bass_utils import run_bass_kernel_spmd

N, D = 4096, 1024
NCORES = 8
R = N // NCORES            # 512 rows per core
P = 128                    # partitions
MI = R // P                # 4 row chunks per core
KC = D // P                # 8 contraction chunks
JW = 512                   # j tile width (one PSUM bank)
NB = 5                     # block-columns per core (self + 4 right neighbors)
JCOLS = NB * JW            # 2560
EPS = 1e-8
BF16 = ml_dtypes.bfloat16
FP8 = ml_dtypes.float8_e4m3
SCALE = 16.0
NWARM = 34

_CACHE = {}


def _build_program():
    nc = bacc.Bacc("TRN2", target_bir_lowering=False, debug=False)
    f32, bf16, fp8 = mybir.dt.float32, mybir.dt.bfloat16, mybir.dt.float8e4
    AF = mybir.ActivationFunctionType
    OP = mybir.AluOpType

    # et packed as k2-pairs: one DMA per pair lands exactly the unit the
    # DoubleRow matmuls consume (5120B per partition per descriptor)
    et_d = nc.dram_tensor("et", [KC // 2, P, 2 * JCOLS], fp8,
                          kind="ExternalInput")
    yb_d = nc.dram_tensor("yb", [P, 1], f32, kind="ExternalInput")
    es_d = nc.dram_tensor("esout", [MI * NB, P, JW], bf16,
                          kind="ExternalOutput")

    with tile.TileContext(nc) as tc, ExitStack() as ctx:
        const = ctx.enter_context(tc.tile_pool(name="const", bufs=1))
        psum = ctx.enter_context(tc.tile_pool(name="psum", bufs=7, space="PSUM"))
        espool = ctx.enter_context(tc.tile_pool(name="es", bufs=8))

        et = const.tile([P, KC, JCOLS], fp8, tag="et")
        b025 = const.tile([P, 1], f32, tag="b025")
        w = const.tile([P, P + 1], bf16, tag="w")
        winit = w[:, 0:1]
        wsrc = w[:, 1:P + 1]
        warm = const.tile([P, 1], f32, tag="warm")

        # Input DMAs: et pairs on the sync ring (they pace the PE, and the
        # ring drains FIFO so earlier pairs land first), labels after; yb
        # tiny on the scalar ring.
        for k2 in range(KC // 2):
            nc.sync.dma_start(out=et[:, 2 * k2:2 * k2 + 2, :], in_=et_d[k2])
        nc.scalar.dma_start(out=b025, in_=yb_d[:])

        nc.vector.memset(w, 1.0)
        # warm the PE clock gate during the initial DMA wait: tiny matmuls
        # into a scratch PSUM bank (reused later by the narrow j=4 tiles)
        wpt = psum.tile([P, JW // 2], f32, tag="pt4", bufs=1)
        for _ in range(NWARM):
            nc.tensor.matmul(
                wpt[96:97, 0:P], winit, wsrc, start=True, stop=True,
                tile_position=(0, 96), skip_group_check=True,
            )
        nc.scalar.activation(warm, b025, AF.Exp, bias=b025, scale=1.0)

        def gram(pt, m, j, k2, c0=0, w=JW, start=None, stop=None):
            nc.tensor.matmul(
                pt[:, 0:w],
                et[:, 2 * k2:2 * k2 + 2, m * P:(m + 1) * P],
                et[:, 2 * k2:2 * k2 + 2, j * JW + c0:j * JW + c0 + w],
                start=(k2 == 0) if start is None else start,
                stop=(k2 == KC // 2 - 1) if stop is None else stop,
                perf_mode=mybir.MatmulPerfMode.DoubleRow,
            )

        rings = [nc.sync, nc.scalar]

        def expmask(pt, m, j, c0=0, w=JW, eng=None):
            # expS = exp(cos*0.25 + 0.25), shipped unmasked to the host,
            # which applies the label mask and does both row and column
            # sums; the DMA engines are idle once the input stream ends.
            es = espool.tile([P, JW], bf16, tag="es")
            nc.scalar.activation(es[:, 0:w], pt[:, 0:w], AF.Exp, bias=b025,
                                 scale=0.25 / (SCALE * SCALE))
            t = m * NB + j
            # early tiles go out on the sync ring, whose FIFO keeps them
            # behind the still-streaming input pairs; late tiles alternate
            # rings so the tail drains in parallel
            ring = rings[t % 2] if t >= 12 else nc.sync
            ring.dma_start(out=es_d[t][:, c0:c0 + w], in_=es[:, 0:w])

        def tile_j(m, j, rev=False, eng=None):
            # one [P, JW] tile: grams then exp+mask. Alternating the k2
            # direction between consecutive tiles makes the boundary
            # LDWEIGHTS identical to its predecessor, which walrus
            # dedupes to ~3ns (instead of an exposed ~130ns load).
            pt = psum.tile([P, JW], f32, tag="pt", name=f"pt_{m}_{j}")
            ks = list(range(KC // 2))[::-1] if rev else list(range(KC // 2))
            for i, k2 in enumerate(ks):
                gram(pt, m, j, k2, start=(i == 0), stop=(i == KC // 2 - 1))
            expmask(pt, m, j, eng=eng)

        # Phase A: m0 k2-major over 5 PSUM banks so the PE consumes et
        # chunk pairs as they stream in during the DMA-paced ramp.
        ptsA = [psum.tile([P, JW], f32, tag="pt", name=f"ptA_{i}")
                for i in range(NB)]
        for k2 in range(KC // 2):
            for j in range(NB):
                gram(ptsA[j], 0, j, k2)
        for j in range(NB):
            expmask(ptsA[j], 0, j)

        # Steady state: per-tile j-inner (back-to-back matmuls into one
        # bank run at full clock; bank completions stagger). Column sums
        # for the first pair of row chunks are emitted once their masks
        # (gated on the late-arriving labels) have had time.
        for idx, (m, j) in enumerate([(1, 0), (1, 1), (1, 2), (1, 3), (1, 4),
                                      (2, 0), (2, 1), (2, 2), (2, 3), (2, 4)]):
            tile_j(m, j, rev=(idx % 2 == 1))

        # m = 3: j=0/4 (no column sums needed) go last, with all column
        # sums emitted before the final tile so the cs eviction overlaps.
        m = MI - 1
        HW2 = JW // 2

        def expmask_half(pt, h):
            # 256-wide exp for one half of the (3,4) block
            esh = espool.tile([P, HW2], bf16, tag="esh", name=f"esh_{h}", bufs=2)
            nc.scalar.activation(esh, pt, AF.Exp,
                                 bias=b025, scale=0.25 / (SCALE * SCALE))
            rings[h].dma_start(out=es_d[MI * NB - 1][:, h * HW2:(h + 1) * HW2],
                               in_=esh)

        tile_j(m, 1, rev=False)
        tile_j(m, 2, rev=True)
        tile_j(m, 3, rev=False)
        # j=4 as two narrow tiles so the serial exp->mask->DMA tail after
        # the last gram matmul is short; the wide j=0 tile sits between
        # them so the halves' shared PSUM bank has time to drain. Row
        # outputs go out on the idle scalar ring so they don't queue
        # behind the last nm DMAs.
        pt4a = psum.tile([P, HW2], f32, tag="pt4", bufs=1)
        for i, k2 in enumerate(reversed(range(KC // 2))):
            nc.tensor.matmul(
                pt4a,
                et[:, 2 * k2:2 * k2 + 2, m * P:(m + 1) * P],
                et[:, 2 * k2:2 * k2 + 2, 4 * JW:4 * JW + HW2],
                start=(i == 0), stop=(i == KC // 2 - 1),
                perf_mode=mybir.MatmulPerfMode.DoubleRow,
            )
        expmask_half(pt4a, 0)
        tile_j(m, 0, rev=False)
        pt4b = psum.tile([P, HW2], f32, tag="pt4", bufs=1)
        for i, k2 in enumerate(reversed(range(KC // 2))):
            nc.tensor.matmul(
                pt4b,
                et[:, 2 * k2:2 * k2 + 2, m * P:(m + 1) * P],
                et[:, 2 * k2:2 * k2 + 2, 4 * JW + HW2:5 * JW],
                start=(i == 0), stop=(i == KC // 2 - 1),
                perf_mode=mybir.MatmulPerfMode.DoubleRow,
            )
        expmask_half(pt4b, 1)

    nc.compile()
    return nc


def _get_program():
    if "nc" not in _CACHE:
        _CACHE["nc"] = _build_program()
    return _CACHE["nc"]


def _host_prep(layer_embeds, y_true):
    E = np.asarray(layer_embeds, dtype=np.float32)
    y = np.asarray(y_true).astype(np.int32)

    norms = np.maximum(np.linalg.norm(E, axis=1), EPS).astype(np.float32)
    Ehf = E / norms[:, None]
    Eh8T = np.ascontiguousarray((Ehf * SCALE).astype(FP8).T)  # [D, N]

    same = y[:, None] == y[None, :]
    nsame = same.sum(1)
    haspos = nsame > 1
    np.fill_diagonal(same, False)
    fp = np.argmax(same, axis=1)                      # first positive (j order)
    posd = np.einsum("ij,ij->i", Ehf, Ehf[fp]).astype(np.float64)
    yb16 = y.astype(BF16)

    in_maps = []
    for c in range(NCORES):
        r0, r1 = c * R, (c + 1) * R
        cols = np.concatenate(
            [np.arange(((c + b) % NCORES) * R, ((c + b) % NCORES) * R + R)
             for b in range(NB)])
        etc = np.ascontiguousarray(
            Eh8T[:, cols].reshape(KC // 2, 2, P, JCOLS)
            .transpose(0, 2, 1, 3).reshape(KC // 2, P, 2 * JCOLS))
        in_maps.append({
            "et": etc,
            "yb": np.full((P, 1), 0.25, dtype=np.float32),
        })
    meta = {"haspos": haspos, "nsame": nsame, "posd": posd, "y": y}
    return in_maps, meta


def _assemble(results, meta):
    """Combine per-core partials into the scalar loss (O(N) host math)."""
    haspos = meta["haspos"]
    nsame = meta["nsame"]
    posd = meta["posd"]

    y = meta["y"]
    neg = np.zeros(N, dtype=np.float64)   # sum over negatives of exp(S)
    for c in range(NCORES):
        r = results[c]
        esv = np.asarray(r["esout"], np.float32)      # [MI*NB, P, JW]
        for m in range(MI):
            rows_m = np.arange(c * R + m * P, c * R + (m + 1) * P)
            yrow = y[rows_m]
            for j in range(NB):
                b = (c + j) % NCORES
                ycol = y[b * R:(b + 1) * R]
                nm = esv[m * NB + j] * (ycol[None, :] != yrow[:, None])
                neg[rows_m] += nm.sum(1, dtype=np.float64)
                if 1 <= j <= 3:
                    # this core is the only one computing the distance
                    # 1..3 blocks; their column sums belong to b's rows
                    neg[b * R:(b + 1) * R] += nm.sum(0, dtype=np.float64)

    posS = (posd + 1.0) * 0.25
    nneg = N - nsame
    total = neg + np.where(haspos, np.exp(posS), 1.0) + (2 * N - 2 - nneg)
    posval = np.where(haspos, posS, 0.0)
    loss = float(np.mean(np.log(total) - posval))
    return np.float32(loss)


def _install_ntff_shim():
    """Provide antenv.axon_hooks (absent in this image) so trace=True works."""
    import importlib
    import types
    try:
        importlib.import_module("antenv.axon_hooks")
        return
    except ImportError:
        pass
    try:
        import antenv
        from trn_agent_boot.trn_boot import _ntff_profile_via_ctypes

        hook = _ntff_profile_via_ctypes("/opt/axon/libaxon_pjrt.so")
        mod = types.ModuleType("antenv.axon_hooks")
        mod._hook = hook
        mod.get_axon_ntff_profile_hook = lambda: mod._hook
        mod.set_axon_ntff_profile_hook = lambda h: setattr(mod, "_hook", h)
        sys.modules["antenv.axon_hooks"] = mod
        antenv.axon_hooks = mod
    except Exception as e:  # profiling is best-effort
        print(f"ntff shim failed: {e}")


def kernel(layer_embeds, y_true, _trace=False):
    import time

    if _trace:
        _install_ntff_shim()
    nc = _get_program()
    in_maps, meta = _host_prep(layer_embeds, y_true)
    last_err = None
    for attempt in range(4):
        try:
            res = run_bass_kernel_spmd(
                nc, in_maps, core_ids=list(range(NCORES)), trace=_trace,
            )
            loss = _assemble(res.results, meta)
            # lse is bounded by log(2N-2) .. log(2N + N*e^0.5) for this
            # problem shape; anything outside is transient corruption.
            if not (np.isfinite(loss) and 5.0 < float(loss) < 20.0):
                raise RuntimeError(f"implausible loss {loss}, retrying")
            if _trace:
                return loss, res
            return loss
        except Exception as e:  # transient device faults: retry
            last_err = e
            time.sleep(5 * (attempt + 1))
    raise last_err


# revision 69
# speedup vs baseline: 1.1504x; 1.1504x over previous
"""Trainium2 Bass kernel for the contrastive loss problem.

Math reformulation of the reference (no [N, 2N-1] scatter needed):
  lse_i = log( exp(pos_val_i) + sum_{j in neg} exp(S_ij) + (2N-2-num_neg_i) )
  loss  = mean_i (lse_i - pos_val_i)
with S = (cos + 1) * 0.25, cos from row-normalized embeddings.

Sharding uses the Gram matrix's symmetry: core c computes only the
[512, 512*5] strip of exp(S) pairing its rows with block-columns
{c, c+1, .., c+4} (mod 8). Columns are pre-rotated on the host so the
program is identical on every core (SPMD). The device does ONLY the
heavy O(N^2 D/8) work per core: stream et in, 80 fp8-DoubleRow gram
matmuls (K=256 per op, x16-prescaled unit rows), exp on ACT, and
stream the 20 exp tiles back out — the label masking and the row /
column sums are O(N^2/8) host adds on the shipped tiles, overlapped
for free since the DMA engines idle once the input stream ends.
Distance-4 blocks are computed by both endpoint cores.

Schedule: the first row chunk is emitted k2-major so the PE consumes
the et pair-DMAs as they stream in; later tiles are j-inner
(back-to-back matmuls into one PSUM bank hold the full PE clock);
warmup matmuls ramp the PE clock during the DMA wait; early exp tiles
go out on the sync ring whose FIFO keeps them behind the input pairs;
the final block is split in two 256-wide halves so the serial
exp->DMA tail is short.

Host: norms, fp8 cast, rotation, first-positive dot products (O(N*D)),
masking + row/column sums of the exp tiles, final assembly.
"""

import sys

sys.path.insert(0, "/opt/trn_rl_repo")

from contextlib import ExitStack

import ml_dtypes
import numpy as np

import concourse.bacc as bacc
import concourse.tile as tile
from concourse import mybir
from concourse.bass_utils import run_bass_kernel_spmd

N, D = 4096, 1024
NCORES = 8
R = N // NCORES            # 512 rows per core
P = 128                    # partitions
MI = R // P                # 4 row chunks per core
KC = D // P                # 8 contraction chunks
JW = 512                   # j tile width (one PSUM bank)
NB = 5                     # block-columns per core (self + 4 right neighbors)
JCOLS = NB * JW            # 2560
EPS = 1e-8
BF16 = ml_dtypes.bfloat16
FP8 = ml_dtypes.float8_e4m3
SCALE = 16.0
NWARM = 38

_CACHE = {}


def _build_program():
    nc = bacc.Bacc("TRN2", target_bir_lowering=False, debug=False)
    f32, bf16, fp8 = mybir.dt.float32, mybir.dt.bfloat16, mybir.dt.float8e4
    AF = mybir.ActivationFunctionType
    OP = mybir.AluOpType

    # et packed as k2-pairs: one DMA per pair lands exactly the unit the
    # DoubleRow matmuls consume (5120B per partition per descriptor)
    et_d = nc.dram_tensor("et", [KC // 2, P, 2 * JCOLS], fp8,
                          kind="ExternalInput")
    yb_d = nc.dram_tensor("yb", [P, 1], f32, kind="ExternalInput")
    es_d = nc.dram_tensor("esout", [MI * NB, P, JW], bf16,
                          kind="ExternalOutput")

    with tile.TileContext(nc) as tc, ExitStack() as ctx:
        const = ctx.enter_context(tc.tile_pool(name="const", bufs=1))
        psum = ctx.enter_context(tc.tile_pool(name="psum", bufs=6, space="PSUM"))
        espool = ctx.enter_context(tc.tile_pool(name="es", bufs=8))

        et = const.tile([P, KC, JCOLS], fp8, tag="et")
        b025 = const.tile([P, 1], f32, tag="b025")
        w = const.tile([P, P + 1], bf16, tag="w")
        winit = w[:, 0:1]
        wsrc = w[:, 1:P + 1]
        warm = const.tile([P, 1], f32, tag="warm")

        # Input DMAs: et pairs on the sync ring (they pace the PE, and the
        # ring drains FIFO so earlier pairs land first), labels after; yb
        # tiny on the scalar ring.
        for k2 in range(KC // 2):
            nc.sync.dma_start(out=et[:, 2 * k2:2 * k2 + 2, :], in_=et_d[k2])
        nc.scalar.dma_start(out=b025, in_=yb_d[:])

        nc.vector.memset(w, 1.0)
        # warm the PE clock gate during the initial DMA wait: tiny matmuls
        # into a scratch PSUM bank (reused later by the narrow j=4 tiles)
        wpt = psum.tile([P, JW // 2], f32, tag="pt4", bufs=2)
        for _ in range(NWARM):
            nc.tensor.matmul(
                wpt[96:97, 0:P], winit, wsrc, start=True, stop=True,
                tile_position=(0, 96), skip_group_check=True,
            )
        nc.scalar.activation(warm, b025, AF.Exp, bias=b025, scale=1.0)

        def gram(pt, m, j, k2, c0=0, w=JW, start=None, stop=None):
            nc.tensor.matmul(
                pt[:, 0:w],
                et[:, 2 * k2:2 * k2 + 2, m * P:(m + 1) * P],
                et[:, 2 * k2:2 * k2 + 2, j * JW + c0:j * JW + c0 + w],
                start=(k2 == 0) if start is None else start,
                stop=(k2 == KC // 2 - 1) if stop is None else stop,
                perf_mode=mybir.MatmulPerfMode.DoubleRow,
            )

        rings = [nc.sync, nc.scalar]

        def expmask(pt, m, j, c0=0, w=JW, eng=None):
            # expS = exp(cos*0.25 + 0.25), shipped unmasked to the host,
            # which applies the label mask and does both row and column
            # sums; the DMA engines are idle once the input stream ends.
            es = espool.tile([P, JW], bf16, tag="es")
            nc.scalar.activation(es[:, 0:w], pt[:, 0:w], AF.Exp, bias=b025,
                                 scale=0.25 / (SCALE * SCALE))
            t = m * NB + j
            # early tiles go out on the sync ring, whose FIFO keeps them
            # behind the still-streaming input pairs; late tiles alternate
            # rings so the tail drains in parallel
            ring = rings[t % 2] if t >= 12 else nc.sync
            ring.dma_start(out=es_d[t][:, c0:c0 + w], in_=es[:, 0:w])

        def tile_j(m, j, rev=False, eng=None):
            # one [P, JW] tile: grams then exp+mask. Alternating the k2
            # direction between consecutive tiles makes the boundary
            # LDWEIGHTS identical to its predecessor, which walrus
            # dedupes to ~3ns (instead of an exposed ~130ns load).
            pt = psum.tile([P, JW], f32, tag="pt", name=f"pt_{m}_{j}")
            ks = list(range(KC // 2))[::-1] if rev else list(range(KC // 2))
            for i, k2 in enumerate(ks):
                gram(pt, m, j, k2, start=(i == 0), stop=(i == KC // 2 - 1))
            expmask(pt, m, j, eng=eng)

        # Phase A: m0 k2-major over 5 PSUM banks so the PE consumes et
        # chunk pairs as they stream in during the DMA-paced ramp.
        ptsA = [psum.tile([P, JW], f32, tag="pt", name=f"ptA_{i}")
                for i in range(NB)]
        for k2 in range(KC // 2):
            for j in range(NB):
                gram(ptsA[j], 0, j, k2)
        for j in range(NB):
            expmask(ptsA[j], 0, j)

        # Steady state: per-tile j-inner (back-to-back matmuls into one
        # bank run at full clock; bank completions stagger). Column sums
        # for the first pair of row chunks are emitted once their masks
        # (gated on the late-arriving labels) have had time.
        for idx, (m, j) in enumerate([(1, 0), (1, 1), (1, 2), (1, 3), (1, 4),
                                      (2, 0), (2, 1), (2, 2), (2, 3), (2, 4)]):
            tile_j(m, j, rev=(idx % 2 == 1))

        # m = 3: j=0/4 (no column sums needed) go last, with all column
        # sums emitted before the final tile so the cs eviction overlaps.
        m = MI - 1
        HW2 = JW // 2

        def expmask_half(pt, h):
            # 256-wide exp for one half of the (3,4) block
            esh = espool.tile([P, HW2], bf16, tag="esh", name=f"esh_{h}", bufs=2)
            nc.scalar.activation(esh, pt, AF.Exp,
                                 bias=b025, scale=0.25 / (SCALE * SCALE))
            rings[h].dma_start(out=es_d[MI * NB - 1][:, h * HW2:(h + 1) * HW2],
                               in_=esh)

        tile_j(m, 1, rev=False)
        tile_j(m, 2, rev=True)
        tile_j(m, 3, rev=False)
        # j=4 as two narrow tiles so the serial exp->mask->DMA tail after
        # the last gram matmul is short; the wide j=0 tile sits between
        # them so the halves' shared PSUM bank has time to drain. Row
        # outputs go out on the idle scalar ring so they don't queue
        # behind the last nm DMAs.
        pt4a = psum.tile([P, HW2], f32, tag="pt4", bufs=2)
        for i, k2 in enumerate(reversed(range(KC // 2))):
            nc.tensor.matmul(
                pt4a,
                et[:, 2 * k2:2 * k2 + 2, m * P:(m + 1) * P],
                et[:, 2 * k2:2 * k2 + 2, 4 * JW:4 * JW + HW2],
                start=(i == 0), stop=(i == KC // 2 - 1),
                perf_mode=mybir.MatmulPerfMode.DoubleRow,
            )
        expmask_half(pt4a, 0)
        tile_j(m, 0, rev=False)
        pt4b = psum.tile([P, HW2], f32, tag="pt4", bufs=2)
        for i, k2 in enumerate(reversed(range(KC // 2))):
            nc.tensor.matmul(
                pt4b,
                et[:, 2 * k2:2 * k2 + 2, m * P:(m + 1) * P],
                et[:, 2 * k2:2 * k2 + 2, 4 * JW + HW2:5 * JW],
                start=(i == 0), stop=(i == KC // 2 - 1),
                perf_mode=mybir.MatmulPerfMode.DoubleRow,
            )
        expmask_half(pt4b, 1)

    nc.compile()
    return nc


def _get_program():
    if "nc" not in _CACHE:
        _CACHE["nc"] = _build_program()
    return _CACHE["nc"]


def _host_prep(layer_embeds, y_true):
    E = np.asarray(layer_embeds, dtype=np.float32)
    y = np.asarray(y_true).astype(np.int32)

    norms = np.maximum(np.linalg.norm(E, axis=1), EPS).astype(np.float32)
    Ehf = E / norms[:, None]
    Eh8T = np.ascontiguousarray((Ehf * SCALE).astype(FP8).T)  # [D, N]

    same = y[:, None] == y[None, :]
    nsame = same.sum(1)
    haspos = nsame > 1
    np.fill_diagonal(same, False)
    fp = np.argmax(same, axis=1)                      # first positive (j order)
    posd = np.einsum("ij,ij->i", Ehf, Ehf[fp]).astype(np.float64)
    yb16 = y.astype(BF16)

    in_maps = []
    for c in range(NCORES):
        r0, r1 = c * R, (c + 1) * R
        cols = np.concatenate(
            [np.arange(((c + b) % NCORES) * R, ((c + b) % NCORES) * R + R)
             for b in range(NB)])
        etc = np.ascontiguousarray(
            Eh8T[:, cols].reshape(KC // 2, 2, P, JCOLS)
            .transpose(0, 2, 1, 3).reshape(KC // 2, P, 2 * JCOLS))
        in_maps.append({
            "et": etc,
            "yb": np.full((P, 1), 0.25, dtype=np.float32),
        })
    meta = {"haspos": haspos, "nsame": nsame, "posd": posd, "y": y}
    return in_maps, meta


def _assemble(results, meta):
    """Combine per-core partials into the scalar loss (O(N) host math)."""
    haspos = meta["haspos"]
    nsame = meta["nsame"]
    posd = meta["posd"]

    y = meta["y"]
    neg = np.zeros(N, dtype=np.float64)   # sum over negatives of exp(S)
    for c in range(NCORES):
        r = results[c]
        esv = np.asarray(r["esout"], np.float32)      # [MI*NB, P, JW]
        for m in range(MI):
            rows_m = np.arange(c * R + m * P, c * R + (m + 1) * P)
            yrow = y[rows_m]
            for j in range(NB):
                b = (c + j) % NCORES
                ycol = y[b * R:(b + 1) * R]
                nm = esv[m * NB + j] * (ycol[None, :] != yrow[:, None])
                neg[rows_m] += nm.sum(1, dtype=np.float64)
                if 1 <= j <= 3:
                    # this core is the only one computing the distance
                    # 1..3 blocks; their column sums belong to b's rows
                    neg[b * R:(b + 1) * R] += nm.sum(0, dtype=np.float64)

    posS = (posd + 1.0) * 0.25
    nneg = N - nsame
    total = neg + np.where(haspos, np.exp(posS), 1.0) + (2 * N - 2 - nneg)
    posval = np.where(haspos, posS, 0.0)
    loss = float(np.mean(np.log(total) - posval))
    return np.float32(loss)


def _install_ntff_shim():
    """Provide antenv.axon_hooks (absent in this image) so trace=True works."""
    import importlib
    import types
    try:
        importlib.import_module("antenv.axon_hooks")
        return
    except ImportError:
        pass
    try:
        import antenv
        from trn_agent_boot.trn_boot import _ntff_profile_via_ctypes

        hook = _ntff_profile_via_ctypes("/opt/axon/libaxon_pjrt.so")
        mod = types.ModuleType("antenv.axon_hooks")
        mod._hook = hook
        mod.get_axon_ntff_profile_hook = lambda: mod._hook
        mod.set_axon_ntff_profile_hook = lambda h: setattr(mod, "_hook", h)
        sys.modules["antenv.axon_hooks"] = mod
        antenv.axon_hooks = mod
    except Exception as e:  # profiling is best-effort
        print(f"ntff shim failed: {e}")


def kernel(layer_embeds, y_true, _trace=False):
    import time

    if _trace:
        _install_ntff_shim()
    nc = _get_program()
    in_maps, meta = _host_prep(layer_embeds, y_true)
    last_err = None
    for attempt in range(4):
        try:
            res = run_bass_kernel_spmd(
                nc, in_maps, core_ids=list(range(NCORES)), trace=_trace,
            )
            loss = _assemble(res.results, meta)
            # lse is bounded by log(2N-2) .. log(2N + N*e^0.5) for this
            # problem shape; anything outside is transient corruption.
            if not (np.isfinite(loss) and 5.0 < float(loss) < 20.0):
                raise RuntimeError(f"implausible loss {loss}, retrying")
            if _trace:
                return loss, res
            return loss
        except Exception as e:  # transient device faults: retry
            last_err = e
            time.sleep(5 * (attempt + 1))
    raise last_err
